# revision 10
# baseline (speedup 1.0000x reference)
"""Distributed Trainium2 kernel for nn_Attention_31370441130243.

Full-input / full-output attention layer, sharded internally over the
8 NeuronCores as (batch=2) x (head-group=4): core c handles batch c//4
and heads [4*(c%4), 4*(c%4)+4).  Each core computes its QKV projections,
per-head RMSNorm + RoPE, non-causal SDPA and a partial output projection
(its Wout column block); the host sums the 4 partials per batch.

v4 design (fp16 compute, fp32 PSUM; evolved from v3):
  - Phase 1 unchanged in structure (shared-stationary QKV, rmsnorm scale
    folded as 1/(rms*C^0.25) into both q and k, deferred PE transposes),
    but the per-head rms scale is ONE broadcast-AP tensor_tensor (not 4
    tensor_scalar ops), the squared-sum uses ACT Square+accum_out, and
    the rope combine writes to a fresh tile (in-place DVE add is ~2.6x
    slower).
  - Phase 2: softmax denominator is OFF the PE matmul stream: e2 tiles
    are summed on DVE (late t-blocks) and GpSimd (early t-blocks), then
    two ones-stationary merge matmuls fold both partial sums into a
    pre-broadcast PSUM denominator.  1/den uses the fast custom-DVE
    reciprocal.  PV matmuls are emitted one t-pair BEHIND the score
    matmuls so the PE never waits on ACT's exp.  The output projection
    of chunk n runs as PE filler inside chunk n+1's score/PV stream.
  - PSUM: scores 2x[128,1024] + po/den ring 3x[128,512] + proj 1.
"""

import math
import sys
from collections import deque

import numpy as np

for _p in ("/opt/trn_rl_repo",):
    if _p not in sys.path:
        sys.path.append(_p)

import bass_rust

import concourse.bass as bass
import concourse.tile as tile
from concourse import mybir
from concourse.bass_utils import run_bass_kernel_spmd
from concourse.masks import make_identity
from concourse.vector_clock import ScopedClock

S, B, D = 2048, 2, 2048
H, C = 16, 128
HL = 4                 # heads per core
M = HL * C             # local qkv rows (512)
EPS = 1e-6
NCORES = 8
ST = S // 128          # 16 s-blocks
DT = D // 128          # 16 d-blocks
NSC = S // 512         # 4 s-chunks for attention
SQRT_C = math.sqrt(C)

f32 = mybir.dt.float32
f16 = mybir.dt.float16
Act = mybir.ActivationFunctionType
Alu = mybir.AluOpType


# ---------------------------------------------------------------------------
# This container's walrus accepts at most one sync-wait command per
# instruction; the stock TileContext exit drain carries one wait per
# outstanding proc.  Split them onto single-wait NoOps.
def _split_drain_and_barrier(self, tick_clock, wait_clock):
    nc = self.nc
    probe = nc.sync.nop(nofuse=True, hint="tile_exit_waits")
    wait_clock.add_sem_waits(probe.ins, ScopedClock({None: tick_clock.global_clock}))
    si = probe.ins.sync_info
    if si is not None and si.on_wait is not None and len(si.on_wait) > 1:
        waits = list(si.on_wait)
        si.on_wait = [waits[0]]
        for w in waits[1:]:
            n2 = nc.sync.nop(nofuse=True, hint="tile_exit_waits")
            n2.ins.sync_info = bass_rust.SyncInfo(on_wait=[w], on_update=[])
    nc.sync.drain(fusable=False)
    nc.all_engine_barrier()
    popped = nc._tile_sem_poison_stack.pop()
    assert popped is self._sem_poison
    nc.clear_and_free_semaphores(list(self.sems.allocated().values()))
    nc.all_engine_barrier()


tile.TileContext._drain_and_barrier = _split_drain_and_barrier


def _split_multi_waits(nc):
    """Walrus here accepts one sync-wait per instruction; hoist extras onto
    single-wait NoOps on the same engine immediately before the instruction."""
    for f in nc.m.functions:
        for bb in f.blocks:
            out = []
            changed = False
            for inst in bb.instructions:
                si = inst.sync_info
                if si is not None and si.on_wait is not None and len(si.on_wait) > 1:
                    waits = list(si.on_wait)
                    si.on_wait = [waits[-1]]
                    for w in waits[:-1]:
                        nop = mybir.InstNoOp(
                            name=f"I-{nc.next_id()}",
                            engine=inst.engine,
                            sync_info=mybir.SyncInfo(on_wait=[w], on_update=[]),
                            bass_nofuse=True,
                        )
                        out.append(nop)
                    changed = True
                out.append(inst)
            if changed:
                bb.instructions[:] = out


def _bcast_heads(ap_2d, heads):
    """View a [128, C] AP as [128, heads, C] with a zero-stride head dim."""
    return bass.AP(
        tensor=ap_2d.tensor,
        offset=ap_2d.offset,
        ap=[ap_2d.ap[0], [0, heads], ap_2d.ap[1]],
    )


def _bcast_inner(ap_2d, inner):
    """View a [128, K] AP as [128, K, inner] with a zero-stride inner dim."""
    return bass.AP(
        tensor=ap_2d.tensor,
        offset=ap_2d.offset,
        ap=[ap_2d.ap[0], ap_2d.ap[1], [0, inner]],
    )


def build_core_kernel(split_waits=True):
    """One core's kernel: partial attention output for 4 heads of 1 batch."""
    nc = bass.Bass()

    # host-prearranged layouts (see make_in_maps):
    #   xp[p, (j, n, c)]  = x[j*128+c_s, n*128+p]   (x^T tiles per s-block)
    #   wq[p, (n, m)]     = Wq_loc[m, n*128+p]
    #   wout[p, (h, e)]   = Wout_loc[e, h*128+p]
    #   cosf/ssinf[p, (j, c)] = table[j*128+p, c]
    xp = nc.declare_dram_parameter("xp", [128, ST * DT * 128], f16, isOutput=False)
    wq = nc.declare_dram_parameter("wq", [128, DT * M], f16, isOutput=False)
    wk = nc.declare_dram_parameter("wk", [128, DT * M], f16, isOutput=False)
    wv = nc.declare_dram_parameter("wv", [128, DT * M], f16, isOutput=False)
    wout = nc.declare_dram_parameter("wout", [128, HL * D], f16, isOutput=False)
    cosf = nc.declare_dram_parameter("cosf", [128, ST * C], f16, isOutput=False)
    ssinf = nc.declare_dram_parameter("ssinf", [128, ST * C], f16, isOutput=False)
    qs = nc.declare_dram_parameter("qs", [C], f16, isOutput=False)
    ks = nc.declare_dram_parameter("ks", [C], f16, isOutput=False)
    out = nc.declare_dram_parameter("out", [S, D], f16, isOutput=True)

    xp_r = xp.rearrange("p (j n c) -> p j n c", j=ST, n=DT)
    wq_r = wq.rearrange("p (n m) -> p n m", n=DT)
    wk_r = wk.rearrange("p (n m) -> p n m", n=DT)
    wv_r = wv.rearrange("p (n m) -> p n m", n=DT)
    wout_r = wout.rearrange("p (h e) -> p h e", h=HL)
    cos_r = cosf.rearrange("p (j c) -> p j c", j=ST)
    ssin_r = ssinf.rearrange("p (j c) -> p j c", j=ST)

    with tile.TileContext(nc) as tc:
        with (
            tc.tile_pool(name="const", bufs=1) as constp,
            tc.tile_pool(name="qkt", bufs=1) as qktp,
            tc.tile_pool(name="vpool", bufs=1) as vpool,
            tc.tile_pool(name="woutp", bufs=1) as woutp,
        ):
            # ---- constants ----
            ident = constp.tile([128, 128], f16, name="ident")
            make_identity(nc, ident)
            ones16 = constp.tile([128, 128], f16, name="ones16")
            nc.vector.memset(ones16, 1.0)
            # bias for rms' = sqrt(ssq/sqrt(C) + eps*sqrt(C)) = rms*C^0.25
            epsb = constp.tile([128, 1], f32, name="epsb")
            nc.vector.memset(epsb, EPS * SQRT_C)

            # qs/ks scale tiles: allocated here, DMA'd after the first
            # weight group so weight bytes hit the DMA pipe first
            qs_bc = constp.tile([128, C], f16, name="qs_bc")
            ks_bc = constp.tile([128, C], f16, name="ks_bc")
            qs_rot = constp.tile([128, C], f16, name="qs_rot")
            ks_rot = constp.tile([128, C], f16, name="ks_rot")

            qT = qktp.tile([128, HL, S], f16, name="qT")
            kT = qktp.tile([128, HL, S], f16, name="kT")
            v_sb = vpool.tile([128, ST, M], f16, name="v_sb")
            wout_sb = woutp.tile([128, HL, D], f16, name="wout_sb")

            # ---- phase 1: QKV projection + rmsnorm + rope + transpose ----
            with (
                tc.tile_pool(name="wqkv", bufs=1) as wqkvp,
                tc.tile_pool(name="rope", bufs=1) as ropep,
                tc.tile_pool(name="ph1", bufs=2) as ph1,
                tc.tile_pool(name="accps", bufs=6, space="PSUM") as accps,
                tc.tile_pool(name="tps", bufs=2, space="PSUM") as tps,
            ):
                wq_sb = wqkvp.tile([128, DT, M], f16, name="wq_sb")
                wk_sb = wqkvp.tile([128, DT, M], f16, name="wk_sb")
                wv_sb = wqkvp.tile([128, DT, M], f16, name="wv_sb")
                # DMA schedule: small first n-group so the first matmul can
                # start early; x tiles and rope tables interleaved; wout and
                # remaining x tiles stream during the j loop.
                WGRPS = [(0, 1), (1, 3), (3, 6), (6, 10), (10, 16)]
                for w_sb, w_r in ((wq_sb, wq_r), (wk_sb, wk_r), (wv_sb, wv_r)):
                    lo, hi = WGRPS[0]
                    nc.sync.dma_start(out=w_sb[:, lo:hi, :], in_=w_r[:, lo:hi, :])
                xjs = {}
                for j in (0, 1):
                    xj = ph1.tile(
                        [128, DT, 128], f16, name="xj", tag="xj", bufs=6
                    )
                    if j == 0:
                        # split so the first matmul gates on n=0..3 only
                        nc.sync.dma_start(
                            out=xj[:, 0:4, :], in_=xp_r[:, j, 0:4, :]
                        )
                        nc.sync.dma_start(
                            out=xj[:, 4:DT, :], in_=xp_r[:, j, 4:DT, :]
                        )
                    else:
                        nc.sync.dma_start(out=xj, in_=xp_r[:, j, :, :])
                    xjs[j] = xj
                for w_bc, w_dram in ((qs_bc, qs), (ks_bc, ks)):
                    src = bass.AP(
                        tensor=w_dram.ap().tensor, offset=0, ap=[[0, 128], [1, C]]
                    )
                    nc.sync.dma_start(out=w_bc, in_=src)
                for w_rot, w_bc in ((qs_rot, qs_bc), (ks_rot, ks_bc)):
                    nc.gpsimd.tensor_copy(
                        out=w_rot[:, 0 : C // 2], in_=w_bc[:, C // 2 : C]
                    )
                    nc.gpsimd.tensor_copy(
                        out=w_rot[:, C // 2 : C], in_=w_bc[:, 0 : C // 2]
                    )
                # PE warmup on resident constants while the weight DMAs
                # stream: ~5us of dummy matmuls gets HAM to 8/8 before the
                # first real matmul instead of paying the cold clock on it
                warm = accps.tile([128, 512], f32, name="warm", tag="acc")
                for _ in range(72):
                    nc.tensor.matmul(
                        warm[:, 0:128], lhsT=ident, rhs=ones16,
                        start=True, stop=True,
                    )
                # prime the ACT Sqrt table while ACT is idle so the first
                # rms sqrt doesn't eat a table load (Exp is primed at the
                # end of the j loop, just before phase 2 needs it)
                dummy = ph1.tile([1, 2], f16, name="dummy", bufs=1)
                nc.scalar.activation(out=dummy[:, 0:1], in_=epsb[0:1, :],
                                     func=Act.Sqrt)
                cos_t = ropep.tile([128, ST, C], f16, name="cos_t")
                ssin_t = ropep.tile([128, ST, C], f16, name="ssin_t")
                for gi, (lo, hi) in enumerate(WGRPS[1:]):
                    for w_sb, w_r in ((wq_sb, wq_r), (wk_sb, wk_r), (wv_sb, wv_r)):
                        nc.sync.dma_start(
                            out=w_sb[:, lo:hi, :], in_=w_r[:, lo:hi, :]
                        )
                    if gi == 0:
                        nc.sync.dma_start(out=cos_t, in_=cos_r)
                        nc.sync.dma_start(out=ssin_t, in_=ssin_r)

                pend_tr = []  # deferred transposes: (t1, dstT, j)

                def flush_transposes(upto_j=None):
                    while pend_tr and (
                        upto_j is None or pend_tr[0][2] <= upto_j
                    ):
                        t1, dstT, j = pend_tr.pop(0)
                        pt = tps.tile([128, M], f16, name="pt")
                        for h in range(HL):
                            nc.tensor.transpose(
                                pt[:, h * C : (h + 1) * C], t1[:, h, :], ident
                            )
                        nc.vector.tensor_copy(
                            out=dstT[:, :, j * 128 : (j + 1) * 128],
                            in_=pt.rearrange("p (a c) -> p a c", a=HL),
                        )

                for j in range(ST):
                    if j in xjs:
                        xj = xjs.pop(j)
                    else:
                        xj = ph1.tile(
                            [128, DT, 128], f16, name="xj", tag="xj", bufs=6
                        )
                        nc.sync.dma_start(out=xj, in_=xp_r[:, j, :, :])
                    if j == 3:
                        for h in range(HL):
                            nc.sync.dma_start(
                                out=wout_sb[:, h, :], in_=wout_r[:, h, :]
                            )
                    # rope tables for this j on GpSimd, ahead of the chain
                    ropes = {}
                    for key, tab, w_bc, w_rot in (
                        ("q", cos_t, qs_bc, qs_rot),
                        ("k", cos_t, ks_bc, ks_rot),
                    ):
                        cwF = ph1.tile([128, M], f16, name="cwF", bufs=4, tag="cwF")
                        nc.gpsimd.tensor_mul(
                            out=cwF.rearrange("p (a c) -> p a c", a=HL),
                            in0=_bcast_heads(cos_t[:, j, :], HL),
                            in1=_bcast_heads(w_bc, HL),
                        )
                        swF = ph1.tile([128, M], f16, name="swF", bufs=4, tag="swF")
                        nc.gpsimd.tensor_mul(
                            out=swF.rearrange("p (a c) -> p a c", a=HL),
                            in0=_bcast_heads(ssin_t[:, j, :], HL),
                            in1=_bcast_heads(w_rot, HL),
                        )
                        ropes[key] = (cwF, swF)
                    pq = accps.tile([128, M], f32, name="pq", tag="acc")
                    pk = accps.tile([128, M], f32, name="pk", tag="acc")
                    pv = accps.tile([128, M], f32, name="pv", tag="acc")
                    for n in range(DT):
                        if n == 12:
                            flush_transposes(upto_j=j - 2)
                        xsl = xj[:, n, :]
                        nc.tensor.matmul(
                            pq, lhsT=xsl, rhs=wq_sb[:, n, :],
                            start=(n == 0), stop=(n == DT - 1),
                        )
                        nc.tensor.matmul(
                            pk, lhsT=xsl, rhs=wk_sb[:, n, :],
                            start=(n == 0), stop=(n == DT - 1),
                        )
                        nc.tensor.matmul(
                            pv, lhsT=xsl, rhs=wv_sb[:, n, :],
                            start=(n == 0), stop=(n == DT - 1),
                        )
                    # v: plain copy to SBUF (cast fp16)
                    nc.scalar.copy(out=v_sb[:, j, :], in_=pv)
                    # q, k: rmsnorm scale + rope, all heads at once
                    for pacc, key, dstT in (
                        (pq, "q", qT),
                        (pk, "k", kT),
                    ):
                        cwF, swF = ropes[key]
                        xq = ph1.tile([128, M], f16, name="xq", bufs=4)
                        nc.scalar.copy(out=xq, in_=pacc)
                        # per-head sum of squares via ACT Square+accum_out
                        ssq4 = ph1.tile([128, HL], f32, name="ssq4", bufs=3)
                        sqd = ph1.tile([128, C], f16, name="sqd", bufs=2)
                        for h in range(HL):
                            nc.scalar.activation(
                                out=sqd, in_=xq[:, h * C : (h + 1) * C],
                                func=Act.Square,
                                accum_out=ssq4[:, h : h + 1],
                            )
                        rms4 = ph1.tile([128, HL], f32, name="rms4", bufs=3)
                        nc.scalar.activation(
                            out=rms4, in_=ssq4,
                            func=Act.Sqrt, scale=1.0 / SQRT_C, bias=epsb,
                        )
                        r4 = ph1.tile([128, HL], f16, name="r4", bufs=3)
                        with nc.allow_low_precision(
                            reason="1/rms fp16: 2e-4 rel on 2e-2 budget"
                        ):
                            nc.vector.reciprocal(out=r4, in_=rms4)
                        # per-head 1/(rms*C^0.25): one broadcast-AP multiply
                        xqs = ph1.tile([128, M], f16, name="xqs", bufs=3)
                        nc.vector.tensor_mul(
                            out=xqs.rearrange("p (a c) -> p a c", a=HL),
                            in0=xq.rearrange("p (a c) -> p a c", a=HL),
                            in1=_bcast_inner(r4, C),
                        )
                        # rotate_half
                        xqs3 = xqs.rearrange("p (a c) -> p a c", a=HL)
                        sh = ph1.tile([128, HL, C], f16, name="sh", bufs=3)
                        nc.vector.tensor_copy(
                            out=sh[:, :, 0 : C // 2], in_=xqs3[:, :, C // 2 : C]
                        )
                        nc.vector.tensor_copy(
                            out=sh[:, :, C // 2 : C], in_=xqs3[:, :, 0 : C // 2]
                        )
                        ta = ph1.tile([128, M], f16, name="ta", bufs=3)
                        shf = sh.rearrange("p a c -> p (a c)")
                        nc.vector.tensor_mul(out=ta, in0=xqs, in1=cwF)
                        nc.vector.tensor_mul(out=shf, in0=shf, in1=swF)
                        t1 = ph1.tile([128, HL, C], f16, name="t1", bufs=6)
                        nc.vector.tensor_add(
                            out=t1.rearrange("p a c -> p (a c)"), in0=ta, in1=shf
                        )
                        pend_tr.append((t1, dstT, j))
                    if j == ST - 1:
                        # preload the Exp/Ln table while the PE finishes the
                        # last block so phase 2's first exp starts clean
                        nc.scalar.activation(
                            out=dummy[:, 1:2], in_=epsb[0:1, :], func=Act.Exp
                        )
                flush_transposes()

            # ---- phase 2: attention + output projection ----
            with (
                tc.tile_pool(name="att", bufs=4) as attp,
                tc.tile_pool(name="esum", bufs=1) as esump,
                tc.tile_pool(name="outT", bufs=2) as outTp,
                tc.tile_pool(name="osb", bufs=1) as osbp,
                tc.tile_pool(name="scps", bufs=2, space="PSUM") as scps,
                tc.tile_pool(name="pod", bufs=3, space="PSUM") as podp,
                tc.tile_pool(name="prj", bufs=1, space="PSUM") as prjp,
            ):
                fillers = deque()   # pending proj-group emitters (PE filler)
                pending_fin = []    # heads awaiting denominator merge

                def emit_filler():
                    if fillers:
                        fillers.popleft()()

                def finish_head(fin):
                    """Merge the DVE+GpSimd partial e-sums into a
                    pre-broadcast PSUM denominator, then fast-reciprocal
                    and scale po into outT."""
                    po, esA, esB, outT_slice = fin
                    den = podp.tile([128, 512], f32, name="den", tag="den",
                                    bufs=1)
                    nc.tensor.matmul(
                        den, lhsT=ones16, rhs=esB, start=True, stop=False
                    )
                    nc.tensor.matmul(
                        den, lhsT=ones16, rhs=esA, start=False, stop=True
                    )
                    # 1/den as exp(-ln(den)) on ACT: 2x~0.7us, vs a 3.3us
                    # DVE reciprocal on [128,512] (custom-DVE approx fails
                    # this walrus' codegen)
                    lden = attp.tile([128, 512], f16, name="lden", bufs=2,
                                     tag="lden")
                    nc.scalar.activation(out=lden, in_=den, func=Act.Ln)
                    rinv = attp.tile([128, 512], f16, name="rinv", bufs=2,
                                     tag="rinv")
                    nc.scalar.activation(out=rinv, in_=lden, func=Act.Exp,
                                         scale=-1.0)
                    nc.vector.tensor_mul(out=outT_slice, in0=po, in1=rinv)

                def make_proj_tasks(outT_prev, nprev, drain=False):
                    for g in range(16):
                        jj, dc = g // 4, g % 4

                        def task(jj=jj, dc=dc, outT_prev=outT_prev,
                                 nprev=nprev, g=g, drain=drain):
                            # in the final drain the po/den ring is idle, so
                            # odd groups borrow it to pipeline 2-wide
                            if drain and g % 2 == 1:
                                psum_out = podp.tile(
                                    [128, 512], f32, name="psum_out",
                                    tag="po", bufs=2
                                )
                            else:
                                psum_out = prjp.tile(
                                    [128, 512], f32, name="psum_out", tag="pr"
                                )
                            for h in range(HL):
                                nc.tensor.matmul(
                                    psum_out,
                                    lhsT=outT_prev[
                                        :, h, jj * 128 : (jj + 1) * 128
                                    ],
                                    rhs=wout_sb[
                                        :, h, dc * 512 : (dc + 1) * 512
                                    ],
                                    start=(h == 0), stop=(h == HL - 1),
                                )
                            out_sb = osbp.tile(
                                [128, 512], f16, name="out_sb", bufs=6,
                                tag="osb"
                            )
                            # alternate cast engine to keep both pipes short
                            if g % 2 == 0:
                                nc.scalar.copy(out=out_sb, in_=psum_out)
                            else:
                                nc.vector.tensor_copy(out=out_sb, in_=psum_out)
                            srow = (nprev * 4 + jj) * 128
                            nc.sync.dma_start(
                                out=out[srow : srow + 128,
                                        dc * 512 : (dc + 1) * 512],
                                in_=out_sb,
                            )

                        fillers.append(task)

                pend_pv = None  # (po, e2, t0, t1b) PV deferred one t-pair

                def emit_pv():
                    nonlocal pend_pv
                    if pend_pv is None:
                        return
                    po, e2, t0, t1b, h = pend_pv
                    pend_pv = None
                    nc.tensor.matmul(
                        po,
                        lhsT=v_sb[:, t0, h * C : (h + 1) * C],
                        rhs=e2[:, 0:512],
                        start=(t0 == 0), stop=False,
                    )
                    nc.tensor.matmul(
                        po,
                        lhsT=v_sb[:, t1b, h * C : (h + 1) * C],
                        rhs=e2[:, 512:1024],
                        start=False, stop=(t1b == ST - 1),
                    )

                for nchunk in range(NSC):
                    ssl = slice(nchunk * 512, (nchunk + 1) * 512)
                    outT_n = outTp.tile([128, HL, 512], f16, name="outT_n")
                    for h in range(HL):
                        po = podp.tile([128, 512], f32, name="po",
                                       tag="po", bufs=2)
                        esA = esump.tile([128, 512], f16, name="esA",
                                         bufs=2, tag="esA")
                        esB = esump.tile([128, 512], f16, name="esB",
                                         bufs=2, tag="esB")
                        for tp in range(ST // 2):
                            t0, t1b = 2 * tp, 2 * tp + 1
                            psc = scps.tile(
                                [128, 1024], f32, name="psc", tag="sc"
                            )
                            nc.tensor.matmul(
                                psc[:, 0:512],
                                lhsT=kT[:, h, t0 * 128 : (t0 + 1) * 128],
                                rhs=qT[:, h, ssl],
                                start=True, stop=True,
                            )
                            nc.tensor.matmul(
                                psc[:, 512:1024],
                                lhsT=kT[:, h, t1b * 128 : (t1b + 1) * 128],
                                rhs=qT[:, h, ssl],
                                start=True, stop=True,
                            )
                            # PV of the previous t-pair: exp already done,
                            # so the PE never waits on ACT here
                            emit_pv()
                            if tp == 1 and pending_fin:
                                finish_head(pending_fin.pop())
                            # filler slots: 16 per chunk; none at (h0, tp<2)
                            # because chunk n-1's h3 outT is only merged at
                            # (h0, tp1) and a filler before that would block
                            # the PE queue on its own input
                            if (
                                tp in (2, 4, 6)
                                or (tp == 0 and h > 0)
                                or (tp == 7 and h == HL - 1)
                            ):
                                emit_filler()
                            e2 = attp.tile([128, 1024], f16, name="e2",
                                           bufs=4, tag="e2")
                            nc.scalar.activation(out=e2, in_=psc, func=Act.Exp)
                            pend_pv = (po, e2, t0, t1b, h)
                            # denominator partial sums off the PE:
                            # early t-pairs on GpSimd, late on DVE
                            if tp == 0:
                                nc.gpsimd.tensor_add(
                                    out=esB, in0=e2[:, 0:512],
                                    in1=e2[:, 512:1024],
                                )
                            elif tp in (1, 2):
                                nc.gpsimd.tensor_add(
                                    out=esB, in0=esB, in1=e2[:, 0:512]
                                )
                                nc.gpsimd.tensor_add(
                                    out=esB, in0=esB, in1=e2[:, 512:1024]
                                )
                            elif tp == 3:
                                nc.vector.tensor_add(
                                    out=esA, in0=e2[:, 0:512],
                                    in1=e2[:, 512:1024],
                                )
                            else:
                                nc.vector.tensor_add(
                                    out=esA, in0=esA, in1=e2[:, 0:512]
                                )
                                nc.vector.tensor_add(
                                    out=esA, in0=esA, in1=e2[:, 512:1024]
                                )
                        pending_fin.append((po, esA, esB, outT_n[:, h, :]))
                    make_proj_tasks(outT_n, nchunk, drain=(nchunk == NSC - 1))
                # drain: last PV, last head finish, last chunk's projection
                emit_pv()
                while pending_fin:
                    finish_head(pending_fin.pop())
                while fillers:
                    emit_filler()
    if split_waits:
        _split_multi_waits(nc)
    return nc


_NC_CACHE = {}


def _get_nc():
    if "nc" not in _NC_CACHE:
        _NC_CACHE["nc"] = build_core_kernel()
    return _NC_CACHE["nc"]


def make_in_maps(x, rope_emb, Wq, Wk, Wv, Wout, q_scale, k_scale):
    freqs = rope_emb.reshape(S, C).astype(np.float64)
    cosf = np.cos(freqs)
    sf = np.sin(freqs)
    ssinf = np.concatenate([-sf[:, : C // 2], sf[:, C // 2 :]], axis=1)
    # [p, (j, c)] layout for the rope tables
    cos_p = np.ascontiguousarray(
        cosf.reshape(ST, 128, C).transpose(1, 0, 2).reshape(128, -1), np.float16
    )
    ssin_p = np.ascontiguousarray(
        ssinf.reshape(ST, 128, C).transpose(1, 0, 2).reshape(128, -1), np.float16
    )
    in_maps = []
    for c in range(NCORES):
        b, hg = c // 4, c % 4
        sl = slice(hg * M, (hg + 1) * M)
        x_b = np.asarray(x[:, b, :], np.float16)  # [S, D]
        # xp[p, j, n, sc] = x_b[j*128+sc, n*128+p]
        xp = np.ascontiguousarray(
            x_b.reshape(ST, 128, DT, 128).transpose(3, 0, 2, 1).reshape(128, -1)
        )
        def wlayout(Wl):  # Wl: [M, D] -> [p, (n, m)]
            return np.ascontiguousarray(
                Wl.T.reshape(DT, 128, M).transpose(1, 0, 2).reshape(128, -1),
                np.float16,
            )
        # wout[p, (h, e)] = Wout[:, sl][e, h*128+p]
        wout_l = np.ascontiguousarray(
            Wout[:, sl].T.reshape(HL, 128, D).transpose(1, 0, 2).reshape(128, -1),
            np.float16,
        )
        in_maps.append(
            {
                "xp": xp,
                "wq": wlayout(np.asarray(Wq[sl, :], np.float32)),
                "wk": wlayout(np.asarray(Wk[sl, :], np.float32)),
                "wv": wlayout(np.asarray(Wv[sl, :], np.float32)),
                "wout": wout_l,
                "cosf": cos_p,
                "ssinf": ssin_p,
                "qs": np.ascontiguousarray(q_scale, np.float16),
                "ks": np.ascontiguousarray(k_scale, np.float16),
            }
        )
    return in_maps


def kernel(x, rope_emb, Wq, Wk, Wv, Wout, q_scale, k_scale, **run_kwargs):
    in_maps = make_in_maps(
        np.asarray(x, np.float32),
        np.asarray(rope_emb, np.float32),
        np.asarray(Wq, np.float32),
        np.asarray(Wk, np.float32),
        np.asarray(Wv, np.float32),
        np.asarray(Wout, np.float32),
        np.asarray(q_scale, np.float32),
        np.asarray(k_scale, np.float32),
    )
    nc = _get_nc()
    res = run_bass_kernel_spmd(nc, in_maps, core_ids=list(range(NCORES)), **run_kwargs)
    out = np.zeros((S, B, D), dtype=np.float32)
    for c in range(NCORES):
        out[:, c // 4, :] += np.asarray(res.results[c]["out"], np.float32)
    if run_kwargs.get("trace"):
        kernel.last_result = res
    return out


# revision 15
# speedup vs baseline: 1.0520x; 1.0520x over previous
"""Distributed Trainium2 kernel for nn_Attention_31370441130243.

Full-input / full-output attention layer, sharded internally over the
8 NeuronCores as (batch=2) x (head-group=4): core c handles batch c//4
and heads [4*(c%4), 4*(c%4)+4).  Each core computes its QKV projections,
per-head RMSNorm + RoPE, non-causal SDPA and a partial output projection
(its Wout column block); the host sums the 4 partials per batch.

v4 design (fp16 compute, fp32 PSUM; evolved from v3):
  - Phase 1 unchanged in structure (shared-stationary QKV, rmsnorm scale
    folded as 1/(rms*C^0.25) into both q and k, deferred PE transposes),
    but the per-head rms scale is ONE broadcast-AP tensor_tensor (not 4
    tensor_scalar ops), the squared-sum uses ACT Square+accum_out, and
    the rope combine writes to a fresh tile (in-place DVE add is ~2.6x
    slower).
  - Phase 2: softmax denominator is OFF the PE matmul stream: e2 tiles
    are summed on DVE (late t-blocks) and GpSimd (early t-blocks), then
    two ones-stationary merge matmuls fold both partial sums into a
    pre-broadcast PSUM denominator.  1/den uses the fast custom-DVE
    reciprocal.  PV matmuls are emitted one t-pair BEHIND the score
    matmuls so the PE never waits on ACT's exp.  The output projection
    of chunk n runs as PE filler inside chunk n+1's score/PV stream.
  - PSUM: scores 2x[128,1024] + po/den ring 3x[128,512] + proj 1.
"""

import math
import sys
from collections import deque

import numpy as np

for _p in ("/opt/trn_rl_repo",):
    if _p not in sys.path:
        sys.path.append(_p)

import bass_rust

import concourse.bass as bass
import concourse.tile as tile
from concourse import mybir
from concourse.bass_utils import run_bass_kernel_spmd
from concourse.masks import make_identity
from concourse.vector_clock import ScopedClock

S, B, D = 2048, 2, 2048
H, C = 16, 128
HL = 4                 # heads per core
M = HL * C             # local qkv rows (512)
EPS = 1e-6
NCORES = 8
ST = S // 128          # 16 s-blocks
DT = D // 128          # 16 d-blocks
NSC = S // 512         # 4 s-chunks for attention
SQRT_C = math.sqrt(C)

f32 = mybir.dt.float32
f16 = mybir.dt.float16
Act = mybir.ActivationFunctionType
Alu = mybir.AluOpType


# ---------------------------------------------------------------------------
# This container's walrus accepts at most one sync-wait command per
# instruction; the stock TileContext exit drain carries one wait per
# outstanding proc.  Split them onto single-wait NoOps.
def _split_drain_and_barrier(self, tick_clock, wait_clock):
    nc = self.nc
    probe = nc.sync.nop(nofuse=True, hint="tile_exit_waits")
    wait_clock.add_sem_waits(probe.ins, ScopedClock({None: tick_clock.global_clock}))
    si = probe.ins.sync_info
    if si is not None and si.on_wait is not None and len(si.on_wait) > 1:
        waits = list(si.on_wait)
        si.on_wait = [waits[0]]
        for w in waits[1:]:
            n2 = nc.sync.nop(nofuse=True, hint="tile_exit_waits")
            n2.ins.sync_info = bass_rust.SyncInfo(on_wait=[w], on_update=[])
    nc.sync.drain(fusable=False)
    nc.all_engine_barrier()
    popped = nc._tile_sem_poison_stack.pop()
    assert popped is self._sem_poison
    nc.clear_and_free_semaphores(list(self.sems.allocated().values()))
    nc.all_engine_barrier()


tile.TileContext._drain_and_barrier = _split_drain_and_barrier


def _split_multi_waits(nc):
    """Walrus here accepts one sync-wait per instruction; hoist extras onto
    single-wait NoOps on the same engine immediately before the instruction."""
    for f in nc.m.functions:
        for bb in f.blocks:
            out = []
            changed = False
            for inst in bb.instructions:
                si = inst.sync_info
                if si is not None and si.on_wait is not None and len(si.on_wait) > 1:
                    waits = list(si.on_wait)
                    si.on_wait = [waits[-1]]
                    for w in waits[:-1]:
                        nop = mybir.InstNoOp(
                            name=f"I-{nc.next_id()}",
                            engine=inst.engine,
                            sync_info=mybir.SyncInfo(on_wait=[w], on_update=[]),
                            bass_nofuse=True,
                        )
                        out.append(nop)
                    changed = True
                out.append(inst)
            if changed:
                bb.instructions[:] = out


def _bcast_heads(ap_2d, heads):
    """View a [128, C] AP as [128, heads, C] with a zero-stride head dim."""
    return bass.AP(
        tensor=ap_2d.tensor,
        offset=ap_2d.offset,
        ap=[ap_2d.ap[0], [0, heads], ap_2d.ap[1]],
    )


def _bcast_inner(ap_2d, inner):
    """View a [128, K] AP as [128, K, inner] with a zero-stride inner dim."""
    return bass.AP(
        tensor=ap_2d.tensor,
        offset=ap_2d.offset,
        ap=[ap_2d.ap[0], ap_2d.ap[1], [0, inner]],
    )


def build_core_kernel(split_waits=True):
    """One core's kernel: partial attention output for 4 heads of 1 batch."""
    nc = bass.Bass()

    # host-prearranged layouts (see make_in_maps):
    #   xp[p, (j, n, c)]  = x[j*128+c_s, n*128+p]   (x^T tiles per s-block)
    #   wq[p, (n, m)]     = Wq_loc[m, n*128+p]
    #   wout[p, (h, e)]   = Wout_loc[e, h*128+p]
    #   cosf/ssinf[p, (j, c)] = table[j*128+p, c]
    xp = nc.declare_dram_parameter("xp", [128, ST * DT * 128], f16, isOutput=False)
    wq = nc.declare_dram_parameter("wq", [128, DT * M], f16, isOutput=False)
    wk = nc.declare_dram_parameter("wk", [128, DT * M], f16, isOutput=False)
    wv = nc.declare_dram_parameter("wv", [128, DT * M], f16, isOutput=False)
    wout = nc.declare_dram_parameter("wout", [128, HL * D], f16, isOutput=False)
    cosf = nc.declare_dram_parameter("cosf", [128, ST * C], f16, isOutput=False)
    ssinf = nc.declare_dram_parameter("ssinf", [128, ST * C], f16, isOutput=False)
    qs = nc.declare_dram_parameter("qs", [C], f16, isOutput=False)
    ks = nc.declare_dram_parameter("ks", [C], f16, isOutput=False)
    out = nc.declare_dram_parameter("out", [S, D], f16, isOutput=True)

    xp_r = xp.rearrange("p (j n c) -> p j n c", j=ST, n=DT)
    wq_r = wq.rearrange("p (n m) -> p n m", n=DT)
    wk_r = wk.rearrange("p (n m) -> p n m", n=DT)
    wv_r = wv.rearrange("p (n m) -> p n m", n=DT)
    wout_r = wout.rearrange("p (h e) -> p h e", h=HL)
    cos_r = cosf.rearrange("p (j c) -> p j c", j=ST)
    ssin_r = ssinf.rearrange("p (j c) -> p j c", j=ST)

    with tile.TileContext(nc) as tc:
        with (
            tc.tile_pool(name="const", bufs=1) as constp,
            tc.tile_pool(name="qkt", bufs=1) as qktp,
            tc.tile_pool(name="vpool", bufs=1) as vpool,
            tc.tile_pool(name="woutp", bufs=1) as woutp,
        ):
            # ---- constants ----
            ident = constp.tile([128, 128], f16, name="ident")
            make_identity(nc, ident)
            ones16 = constp.tile([128, 128], f16, name="ones16")
            nc.vector.memset(ones16, 1.0)
            # bias for rms' = sqrt(ssq/sqrt(C) + eps*sqrt(C)) = rms*C^0.25
            epsb = constp.tile([128, 1], f32, name="epsb")
            nc.vector.memset(epsb, EPS * SQRT_C)

            # qs/ks scale tiles: allocated here, DMA'd after the first
            # weight group so weight bytes hit the DMA pipe first
            qs_bc = constp.tile([128, C], f16, name="qs_bc")
            ks_bc = constp.tile([128, C], f16, name="ks_bc")
            qs_rot = constp.tile([128, C], f16, name="qs_rot")
            ks_rot = constp.tile([128, C], f16, name="ks_rot")

            qT = qktp.tile([128, HL, S], f16, name="qT")
            kT = qktp.tile([128, HL, S], f16, name="kT")
            v_sb = vpool.tile([128, ST, M], f16, name="v_sb")
            wout_sb = woutp.tile([128, HL, D], f16, name="wout_sb")

            # ---- phase 1: QKV projection + rmsnorm + rope + transpose ----
            with (
                tc.tile_pool(name="wqkv", bufs=1) as wqkvp,
                tc.tile_pool(name="rope", bufs=1) as ropep,
                tc.tile_pool(name="ph1", bufs=2) as ph1,
                tc.tile_pool(name="accps", bufs=6, space="PSUM") as accps,
                tc.tile_pool(name="tps", bufs=2, space="PSUM") as tps,
            ):
                wq_sb = wqkvp.tile([128, DT, M], f16, name="wq_sb")
                wk_sb = wqkvp.tile([128, DT, M], f16, name="wk_sb")
                wv_sb = wqkvp.tile([128, DT, M], f16, name="wv_sb")
                # DMA schedule: small first n-group so the first matmul can
                # start early; x tiles and rope tables interleaved; wout and
                # remaining x tiles stream during the j loop.
                WGRPS = [(0, 1), (1, 3), (3, 6), (6, 10), (10, 16)]
                for w_sb, w_r in ((wq_sb, wq_r), (wk_sb, wk_r), (wv_sb, wv_r)):
                    lo, hi = WGRPS[0]
                    nc.sync.dma_start(out=w_sb[:, lo:hi, :], in_=w_r[:, lo:hi, :])
                xjs = {}
                for j in (0, 1, 2):
                    xj = ph1.tile(
                        [128, DT, 128], f16, name="xj", tag="xj", bufs=6
                    )
                    if j == 0:
                        # split so the first matmul gates on n=0..3 only
                        nc.sync.dma_start(
                            out=xj[:, 0:4, :], in_=xp_r[:, j, 0:4, :]
                        )
                        nc.sync.dma_start(
                            out=xj[:, 4:DT, :], in_=xp_r[:, j, 4:DT, :]
                        )
                    else:
                        nc.sync.dma_start(out=xj, in_=xp_r[:, j, :, :])
                    xjs[j] = xj
                for w_bc, w_dram in ((qs_bc, qs), (ks_bc, ks)):
                    src = bass.AP(
                        tensor=w_dram.ap().tensor, offset=0, ap=[[0, 128], [1, C]]
                    )
                    nc.sync.dma_start(out=w_bc, in_=src)
                for w_rot, w_bc in ((qs_rot, qs_bc), (ks_rot, ks_bc)):
                    nc.gpsimd.tensor_copy(
                        out=w_rot[:, 0 : C // 2], in_=w_bc[:, C // 2 : C]
                    )
                    nc.gpsimd.tensor_copy(
                        out=w_rot[:, C // 2 : C], in_=w_bc[:, 0 : C // 2]
                    )
                # PE warmup on resident constants while the weight DMAs
                # stream: ~5us of dummy matmuls gets HAM to 8/8 before the
                # first real matmul instead of paying the cold clock on it
                warm = accps.tile([128, 512], f32, name="warm", tag="acc")
                for _ in range(72):
                    nc.tensor.matmul(
                        warm[:, 0:128], lhsT=ident, rhs=ones16,
                        start=True, stop=True,
                    )
                # prime the ACT Sqrt table while ACT is idle so the first
                # rms sqrt doesn't eat a table load (Exp is primed at the
                # end of the j loop, just before phase 2 needs it)
                dummy = ph1.tile([1, 2], f16, name="dummy", bufs=1)
                nc.scalar.activation(out=dummy[:, 0:1], in_=epsb[0:1, :],
                                     func=Act.Sqrt)
                cos_t = ropep.tile([128, ST, C], f16, name="cos_t")
                ssin_t = ropep.tile([128, ST, C], f16, name="ssin_t")
                for gi, (lo, hi) in enumerate(WGRPS[1:]):
                    for w_sb, w_r in ((wq_sb, wq_r), (wk_sb, wk_r), (wv_sb, wv_r)):
                        nc.sync.dma_start(
                            out=w_sb[:, lo:hi, :], in_=w_r[:, lo:hi, :]
                        )
                    if gi == 0:
                        nc.sync.dma_start(out=cos_t, in_=cos_r)
                        nc.sync.dma_start(out=ssin_t, in_=ssin_r)

                pend_tr = []  # deferred transposes: (t1, dstT, j)

                def flush_transposes(upto_j=None):
                    while pend_tr and (
                        upto_j is None or pend_tr[0][2] <= upto_j
                    ):
                        t1, dstT, j = pend_tr.pop(0)
                        pt = tps.tile([128, M], f16, name="pt")
                        for h in range(HL):
                            nc.tensor.transpose(
                                pt[:, h * C : (h + 1) * C], t1[:, h, :], ident
                            )
                        nc.vector.tensor_copy(
                            out=dstT[:, :, j * 128 : (j + 1) * 128],
                            in_=pt.rearrange("p (a c) -> p a c", a=HL),
                        )

                for j in range(ST):
                    if j in xjs:
                        xj = xjs.pop(j)
                    else:
                        xj = ph1.tile(
                            [128, DT, 128], f16, name="xj", tag="xj", bufs=6
                        )
                        nc.sync.dma_start(out=xj, in_=xp_r[:, j, :, :])
                    if j == 3:
                        for h in range(HL):
                            nc.sync.dma_start(
                                out=wout_sb[:, h, :], in_=wout_r[:, h, :]
                            )
                    # rope tables for this j on GpSimd, ahead of the chain
                    ropes = {}
                    for key, tab, w_bc, w_rot in (
                        ("q", cos_t, qs_bc, qs_rot),
                        ("k", cos_t, ks_bc, ks_rot),
                    ):
                        cwF = ph1.tile([128, M], f16, name="cwF", bufs=4, tag="cwF")
                        nc.gpsimd.tensor_mul(
                            out=cwF.rearrange("p (a c) -> p a c", a=HL),
                            in0=_bcast_heads(cos_t[:, j, :], HL),
                            in1=_bcast_heads(w_bc, HL),
                        )
                        swF = ph1.tile([128, M], f16, name="swF", bufs=4, tag="swF")
                        nc.gpsimd.tensor_mul(
                            out=swF.rearrange("p (a c) -> p a c", a=HL),
                            in0=_bcast_heads(ssin_t[:, j, :], HL),
                            in1=_bcast_heads(w_rot, HL),
                        )
                        ropes[key] = (cwF, swF)
                    pq = accps.tile([128, M], f32, name="pq", tag="acc")
                    pk = accps.tile([128, M], f32, name="pk", tag="acc")
                    pv = accps.tile([128, M], f32, name="pv", tag="acc")
                    for n in range(DT):
                        if n == 12:
                            flush_transposes(upto_j=j - 2)
                        xsl = xj[:, n, :]
                        nc.tensor.matmul(
                            pq, lhsT=xsl, rhs=wq_sb[:, n, :],
                            start=(n == 0), stop=(n == DT - 1),
                        )
                        nc.tensor.matmul(
                            pk, lhsT=xsl, rhs=wk_sb[:, n, :],
                            start=(n == 0), stop=(n == DT - 1),
                        )
                        nc.tensor.matmul(
                            pv, lhsT=xsl, rhs=wv_sb[:, n, :],
                            start=(n == 0), stop=(n == DT - 1),
                        )
                    # v: plain copy to SBUF (cast fp16)
                    nc.scalar.copy(out=v_sb[:, j, :], in_=pv)
                    # q, k: rmsnorm scale + rope, all heads at once
                    for pacc, key, dstT in (
                        (pq, "q", qT),
                        (pk, "k", kT),
                    ):
                        cwF, swF = ropes[key]
                        xq = ph1.tile([128, M], f16, name="xq", bufs=4)
                        nc.scalar.copy(out=xq, in_=pacc)
                        # per-head sum of squares via ACT Square+accum_out
                        ssq4 = ph1.tile([128, HL], f32, name="ssq4", bufs=3)
                        sqd = ph1.tile([128, C], f16, name="sqd", bufs=2)
                        for h in range(HL):
                            nc.scalar.activation(
                                out=sqd, in_=xq[:, h * C : (h + 1) * C],
                                func=Act.Square,
                                accum_out=ssq4[:, h : h + 1],
                            )
                        rms4 = ph1.tile([128, HL], f32, name="rms4", bufs=3)
                        nc.scalar.activation(
                            out=rms4, in_=ssq4,
                            func=Act.Sqrt, scale=1.0 / SQRT_C, bias=epsb,
                        )
                        r4 = ph1.tile([128, HL], f16, name="r4", bufs=3)
                        with nc.allow_low_precision(
                            reason="1/rms fp16: 2e-4 rel on 2e-2 budget"
                        ):
                            nc.vector.reciprocal(out=r4, in_=rms4)
                        # per-head 1/(rms*C^0.25): one broadcast-AP multiply
                        xqs = ph1.tile([128, M], f16, name="xqs", bufs=3)
                        nc.vector.tensor_mul(
                            out=xqs.rearrange("p (a c) -> p a c", a=HL),
                            in0=xq.rearrange("p (a c) -> p a c", a=HL),
                            in1=_bcast_inner(r4, C),
                        )
                        # rotate_half
                        xqs3 = xqs.rearrange("p (a c) -> p a c", a=HL)
                        sh = ph1.tile([128, HL, C], f16, name="sh", bufs=3)
                        nc.vector.tensor_copy(
                            out=sh[:, :, 0 : C // 2], in_=xqs3[:, :, C // 2 : C]
                        )
                        nc.vector.tensor_copy(
                            out=sh[:, :, C // 2 : C], in_=xqs3[:, :, 0 : C // 2]
                        )
                        ta = ph1.tile([128, M], f16, name="ta", bufs=3)
                        shf = sh.rearrange("p a c -> p (a c)")
                        nc.vector.tensor_mul(out=ta, in0=xqs, in1=cwF)
                        nc.vector.tensor_mul(out=shf, in0=shf, in1=swF)
                        t1 = ph1.tile([128, HL, C], f16, name="t1", bufs=6)
                        nc.vector.tensor_add(
                            out=t1.rearrange("p a c -> p (a c)"), in0=ta, in1=shf
                        )
                        pend_tr.append((t1, dstT, j))
                    if j == ST - 1:
                        # preload the Exp/Ln table while the PE finishes the
                        # last block so phase 2's first exp starts clean
                        nc.scalar.activation(
                            out=dummy[:, 1:2], in_=epsb[0:1, :], func=Act.Exp
                        )
                flush_transposes()

            # ---- phase 2: attention + output projection ----
            with (
                tc.tile_pool(name="att", bufs=4) as attp,
                tc.tile_pool(name="esum", bufs=1) as esump,
                tc.tile_pool(name="outT", bufs=2) as outTp,
                tc.tile_pool(name="osb", bufs=1) as osbp,
                tc.tile_pool(name="scps", bufs=2, space="PSUM") as scps,
                tc.tile_pool(name="pod", bufs=3, space="PSUM") as podp,
                tc.tile_pool(name="prj", bufs=1, space="PSUM") as prjp,
            ):
                fillers = deque()   # pending proj-group emitters (PE filler)
                pending_fin = []    # heads awaiting denominator merge

                def emit_filler():
                    if fillers:
                        fillers.popleft()()

                def finish_head(fin):
                    """Merge the DVE+GpSimd partial e-sums into a
                    pre-broadcast PSUM denominator, then fast-reciprocal
                    and scale po into outT."""
                    po, esA, esB, outT_slice = fin
                    den = podp.tile([128, 512], f32, name="den", tag="den",
                                    bufs=1)
                    nc.tensor.matmul(
                        den, lhsT=ones16, rhs=esB, start=True, stop=False
                    )
                    nc.tensor.matmul(
                        den, lhsT=ones16, rhs=esA, start=False, stop=True
                    )
                    # 1/den on DVE: slow (~3.3us) but emitted in DVE's idle
                    # window (tp2, before the den adds start at tp3) and
                    # nothing downstream needs outT for >10us.  Keeping it
                    # off ACT matters: any non-exp ACT op delays exp(tp) and
                    # stalls the score-PSUM ring two t-pairs later.
                    rinv = attp.tile([128, 512], f16, name="rinv", bufs=2,
                                     tag="rinv")
                    with nc.allow_low_precision(
                        reason="1/denominator fp16: 5e-4 rel on 2e-2 budget"
                    ):
                        nc.vector.reciprocal(out=rinv, in_=den)
                    nc.vector.tensor_mul(out=outT_slice, in0=po, in1=rinv)

                def make_proj_tasks(outT_prev, nprev, drain=False):
                    for g in range(16):
                        jj, dc = g // 4, g % 4

                        def task(jj=jj, dc=dc, outT_prev=outT_prev,
                                 nprev=nprev, g=g, drain=drain):
                            # in the final drain the po/den ring is idle, so
                            # odd groups borrow it to pipeline 2-wide
                            if drain and g % 2 == 1:
                                psum_out = podp.tile(
                                    [128, 512], f32, name="psum_out",
                                    tag="po", bufs=2
                                )
                            else:
                                psum_out = prjp.tile(
                                    [128, 512], f32, name="psum_out", tag="pr"
                                )
                            for h in range(HL):
                                nc.tensor.matmul(
                                    psum_out,
                                    lhsT=outT_prev[
                                        :, h, jj * 128 : (jj + 1) * 128
                                    ],
                                    rhs=wout_sb[
                                        :, h, dc * 512 : (dc + 1) * 512
                                    ],
                                    start=(h == 0), stop=(h == HL - 1),
                                )
                            out_sb = osbp.tile(
                                [128, 512], f16, name="out_sb", bufs=6,
                                tag="osb"
                            )
                            # GpSimd cannot read PSUM; split casts ACT/DVE
                            # (2 each per head -- fits both engines' slack)
                            if g % 2 == 0:
                                nc.scalar.copy(out=out_sb, in_=psum_out)
                            else:
                                nc.vector.tensor_copy(out=out_sb, in_=psum_out)
                            srow = (nprev * 4 + jj) * 128
                            nc.sync.dma_start(
                                out=out[srow : srow + 128,
                                        dc * 512 : (dc + 1) * 512],
                                in_=out_sb,
                            )

                        fillers.append(task)

                pend_pv = None  # (po, e2, t0, t1b) PV deferred one t-pair

                def emit_pv():
                    nonlocal pend_pv
                    if pend_pv is None:
                        return
                    po, e2, t0, t1b, h = pend_pv
                    pend_pv = None
                    nc.tensor.matmul(
                        po,
                        lhsT=v_sb[:, t0, h * C : (h + 1) * C],
                        rhs=e2[:, 0:512],
                        start=(t0 == 0), stop=False,
                    )
                    nc.tensor.matmul(
                        po,
                        lhsT=v_sb[:, t1b, h * C : (h + 1) * C],
                        rhs=e2[:, 512:1024],
                        start=False, stop=(t1b == ST - 1),
                    )

                for nchunk in range(NSC):
                    ssl = slice(nchunk * 512, (nchunk + 1) * 512)
                    outT_n = outTp.tile([128, HL, 512], f16, name="outT_n")
                    for h in range(HL):
                        po = podp.tile([128, 512], f32, name="po",
                                       tag="po", bufs=2)
                        esA = esump.tile([128, 512], f16, name="esA",
                                         bufs=2, tag="esA")
                        esB = esump.tile([128, 512], f16, name="esB",
                                         bufs=2, tag="esB")
                        for tp in range(ST // 2):
                            t0, t1b = 2 * tp, 2 * tp + 1
                            psc = scps.tile(
                                [128, 1024], f32, name="psc", tag="sc"
                            )
                            nc.tensor.matmul(
                                psc[:, 0:512],
                                lhsT=kT[:, h, t0 * 128 : (t0 + 1) * 128],
                                rhs=qT[:, h, ssl],
                                start=True, stop=True,
                            )
                            nc.tensor.matmul(
                                psc[:, 512:1024],
                                lhsT=kT[:, h, t1b * 128 : (t1b + 1) * 128],
                                rhs=qT[:, h, ssl],
                                start=True, stop=True,
                            )
                            # PV of the previous t-pair: exp already done,
                            # so the PE never waits on ACT here
                            emit_pv()
                            if tp == 2 and pending_fin:
                                finish_head(pending_fin.pop())
                            # filler slots: >=16 per chunk; none at (h0,
                            # tp<3) because chunk n-1's h3 outT is only
                            # merged at (h0, tp2) and a filler before that
                            # would block the PE queue on its own input
                            if (
                                tp in (3, 5, 7)
                                or (tp in (0, 1) and h > 0)
                            ):
                                emit_filler()
                            e2 = attp.tile([128, 1024], f16, name="e2",
                                           bufs=4, tag="e2")
                            nc.scalar.activation(out=e2, in_=psc, func=Act.Exp)
                            pend_pv = (po, e2, t0, t1b, h)
                            # denominator partial sums off the PE:
                            # early t-pairs on GpSimd, late on DVE
                            if tp == 0:
                                nc.gpsimd.tensor_add(
                                    out=esB, in0=e2[:, 0:512],
                                    in1=e2[:, 512:1024],
                                )
                            elif tp in (1, 2):
                                nc.gpsimd.tensor_add(
                                    out=esB, in0=esB, in1=e2[:, 0:512]
                                )
                                nc.gpsimd.tensor_add(
                                    out=esB, in0=esB, in1=e2[:, 512:1024]
                                )
                            elif tp == 3:
                                nc.vector.tensor_add(
                                    out=esA, in0=e2[:, 0:512],
                                    in1=e2[:, 512:1024],
                                )
                            else:
                                nc.vector.tensor_add(
                                    out=esA, in0=esA, in1=e2[:, 0:512]
                                )
                                nc.vector.tensor_add(
                                    out=esA, in0=esA, in1=e2[:, 512:1024]
                                )
                        pending_fin.append((po, esA, esB, outT_n[:, h, :]))
                    make_proj_tasks(outT_n, nchunk, drain=(nchunk == NSC - 1))
                # drain: last PV, last head finish, last chunk's projection
                emit_pv()
                while pending_fin:
                    finish_head(pending_fin.pop())
                while fillers:
                    emit_filler()
    if split_waits:
        _split_multi_waits(nc)
    return nc


_NC_CACHE = {}


def _get_nc():
    if "nc" not in _NC_CACHE:
        _NC_CACHE["nc"] = build_core_kernel()
    return _NC_CACHE["nc"]


def make_in_maps(x, rope_emb, Wq, Wk, Wv, Wout, q_scale, k_scale):
    freqs = rope_emb.reshape(S, C).astype(np.float64)
    cosf = np.cos(freqs)
    sf = np.sin(freqs)
    ssinf = np.concatenate([-sf[:, : C // 2], sf[:, C // 2 :]], axis=1)
    # [p, (j, c)] layout for the rope tables
    cos_p = np.ascontiguousarray(
        cosf.reshape(ST, 128, C).transpose(1, 0, 2).reshape(128, -1), np.float16
    )
    ssin_p = np.ascontiguousarray(
        ssinf.reshape(ST, 128, C).transpose(1, 0, 2).reshape(128, -1), np.float16
    )
    in_maps = []
    for c in range(NCORES):
        b, hg = c // 4, c % 4
        sl = slice(hg * M, (hg + 1) * M)
        x_b = np.asarray(x[:, b, :], np.float16)  # [S, D]
        # xp[p, j, n, sc] = x_b[j*128+sc, n*128+p]
        xp = np.ascontiguousarray(
            x_b.reshape(ST, 128, DT, 128).transpose(3, 0, 2, 1).reshape(128, -1)
        )
        def wlayout(Wl):  # Wl: [M, D] -> [p, (n, m)]
            return np.ascontiguousarray(
                Wl.T.reshape(DT, 128, M).transpose(1, 0, 2).reshape(128, -1),
                np.float16,
            )
        # wout[p, (h, e)] = Wout[:, sl][e, h*128+p]
        wout_l = np.ascontiguousarray(
            Wout[:, sl].T.reshape(HL, 128, D).transpose(1, 0, 2).reshape(128, -1),
            np.float16,
        )
        in_maps.append(
            {
                "xp": xp,
                "wq": wlayout(np.asarray(Wq[sl, :], np.float32)),
                "wk": wlayout(np.asarray(Wk[sl, :], np.float32)),
                "wv": wlayout(np.asarray(Wv[sl, :], np.float32)),
                "wout": wout_l,
                "cosf": cos_p,
                "ssinf": ssin_p,
                "qs": np.ascontiguousarray(q_scale, np.float16),
                "ks": np.ascontiguousarray(k_scale, np.float16),
            }
        )
    return in_maps


def kernel(x, rope_emb, Wq, Wk, Wv, Wout, q_scale, k_scale, **run_kwargs):
    in_maps = make_in_maps(
        np.asarray(x, np.float32),
        np.asarray(rope_emb, np.float32),
        np.asarray(Wq, np.float32),
        np.asarray(Wk, np.float32),
        np.asarray(Wv, np.float32),
        np.asarray(Wout, np.float32),
        np.asarray(q_scale, np.float32),
        np.asarray(k_scale, np.float32),
    )
    nc = _get_nc()
    res = run_bass_kernel_spmd(nc, in_maps, core_ids=list(range(NCORES)), **run_kwargs)
    out = np.zeros((S, B, D), dtype=np.float32)
    for c in range(NCORES):
        out[:, c // 4, :] += np.asarray(res.results[c]["out"], np.float32)
    if run_kwargs.get("trace"):
        kernel.last_result = res
    return out


# revision 22
# speedup vs baseline: 1.0705x; 1.0175x over previous
"""Distributed Trainium2 kernel for nn_Attention_31370441130243.

Full-input / full-output attention layer, sharded internally over the
8 NeuronCores as (batch=2) x (head-group=4): core c handles batch c//4
and heads [4*(c%4), 4*(c%4)+4).  Each core computes its QKV projections,
per-head RMSNorm + RoPE, non-causal SDPA and a partial output projection
(its Wout column block); the host sums the 4 partials per batch.

v4 design (fp16 compute, fp32 PSUM; evolved from v3):
  - Phase 1 unchanged in structure (shared-stationary QKV, rmsnorm scale
    folded as 1/(rms*C^0.25) into both q and k, deferred PE transposes),
    but the per-head rms scale is ONE broadcast-AP tensor_tensor (not 4
    tensor_scalar ops), the squared-sum uses ACT Square+accum_out, and
    the rope combine writes to a fresh tile (in-place DVE add is ~2.6x
    slower).
  - Phase 2: softmax denominator is OFF the PE matmul stream: e2 tiles
    are summed on DVE (late t-blocks) and GpSimd (early t-blocks), then
    two ones-stationary merge matmuls fold both partial sums into a
    pre-broadcast PSUM denominator.  1/den uses the fast custom-DVE
    reciprocal.  PV matmuls are emitted one t-pair BEHIND the score
    matmuls so the PE never waits on ACT's exp.  The output projection
    of chunk n runs as PE filler inside chunk n+1's score/PV stream.
  - PSUM: scores 2x[128,1024] + po/den ring 3x[128,512] + proj 1.
"""

import math
import sys
from collections import deque

import numpy as np

for _p in ("/opt/trn_rl_repo",):
    if _p not in sys.path:
        sys.path.append(_p)

import bass_rust

import concourse.bass as bass
import concourse.tile as tile
from concourse import mybir
from concourse.bass_utils import run_bass_kernel_spmd
from concourse.masks import make_identity
from concourse.vector_clock import ScopedClock

S, B, D = 2048, 2, 2048
H, C = 16, 128
HL = 4                 # heads per core
M = HL * C             # local qkv rows (512)
EPS = 1e-6
NCORES = 8
ST = S // 128          # 16 s-blocks
DT = D // 128          # 16 d-blocks
NSC = S // 512         # 4 s-chunks for attention
SQRT_C = math.sqrt(C)

f32 = mybir.dt.float32
f16 = mybir.dt.float16
Act = mybir.ActivationFunctionType
Alu = mybir.AluOpType


# ---------------------------------------------------------------------------
# This container's walrus accepts at most one sync-wait command per
# instruction; the stock TileContext exit drain carries one wait per
# outstanding proc.  Split them onto single-wait NoOps.
def _split_drain_and_barrier(self, tick_clock, wait_clock):
    nc = self.nc
    probe = nc.sync.nop(nofuse=True, hint="tile_exit_waits")
    wait_clock.add_sem_waits(probe.ins, ScopedClock({None: tick_clock.global_clock}))
    si = probe.ins.sync_info
    if si is not None and si.on_wait is not None and len(si.on_wait) > 1:
        waits = list(si.on_wait)
        si.on_wait = [waits[0]]
        for w in waits[1:]:
            n2 = nc.sync.nop(nofuse=True, hint="tile_exit_waits")
            n2.ins.sync_info = bass_rust.SyncInfo(on_wait=[w], on_update=[])
    nc.sync.drain(fusable=False)
    nc.all_engine_barrier()
    popped = nc._tile_sem_poison_stack.pop()
    assert popped is self._sem_poison
    nc.clear_and_free_semaphores(list(self.sems.allocated().values()))
    nc.all_engine_barrier()


tile.TileContext._drain_and_barrier = _split_drain_and_barrier


def _split_multi_waits(nc):
    """Walrus here accepts one sync-wait per instruction; hoist extras onto
    single-wait NoOps on the same engine immediately before the instruction."""
    for f in nc.m.functions:
        for bb in f.blocks:
            out = []
            changed = False
            for inst in bb.instructions:
                si = inst.sync_info
                if si is not None and si.on_wait is not None and len(si.on_wait) > 1:
                    waits = list(si.on_wait)
                    si.on_wait = [waits[-1]]
                    for w in waits[:-1]:
                        nop = mybir.InstNoOp(
                            name=f"I-{nc.next_id()}",
                            engine=inst.engine,
                            sync_info=mybir.SyncInfo(on_wait=[w], on_update=[]),
                            bass_nofuse=True,
                        )
                        out.append(nop)
                    changed = True
                out.append(inst)
            if changed:
                bb.instructions[:] = out


def _bcast_heads(ap_2d, heads):
    """View a [128, C] AP as [128, heads, C] with a zero-stride head dim."""
    return bass.AP(
        tensor=ap_2d.tensor,
        offset=ap_2d.offset,
        ap=[ap_2d.ap[0], [0, heads], ap_2d.ap[1]],
    )


def _bcast_inner(ap_2d, inner):
    """View a [128, K] AP as [128, K, inner] with a zero-stride inner dim."""
    return bass.AP(
        tensor=ap_2d.tensor,
        offset=ap_2d.offset,
        ap=[ap_2d.ap[0], ap_2d.ap[1], [0, inner]],
    )


def build_core_kernel(split_waits=True):
    """One core's kernel: partial attention output for 4 heads of 1 batch."""
    nc = bass.Bass()

    # host-prearranged layouts (see make_in_maps):
    #   xp[p, (j, n, c)]  = x[j*128+c_s, n*128+p]   (x^T tiles per s-block)
    #   wq[p, (n, m)]     = Wq_loc[m, n*128+p]
    #   wout[p, (h, e)]   = Wout_loc[e, h*128+p]
    #   cosf/ssinf[p, (j, c)] = table[j*128+p, c]
    xp = nc.declare_dram_parameter("xp", [128, ST * DT * 128], f16, isOutput=False)
    wq = nc.declare_dram_parameter("wq", [128, DT * M], f16, isOutput=False)
    wk = nc.declare_dram_parameter("wk", [128, DT * M], f16, isOutput=False)
    wv = nc.declare_dram_parameter("wv", [128, DT * M], f16, isOutput=False)
    wout = nc.declare_dram_parameter("wout", [128, HL * D], f16, isOutput=False)
    cosf = nc.declare_dram_parameter("cosf", [128, ST * C], f16, isOutput=False)
    ssinf = nc.declare_dram_parameter("ssinf", [128, ST * C], f16, isOutput=False)
    qs = nc.declare_dram_parameter("qs", [C], f16, isOutput=False)
    ks = nc.declare_dram_parameter("ks", [C], f16, isOutput=False)
    out = nc.declare_dram_parameter("out", [S, D], f16, isOutput=True)

    xp_r = xp.rearrange("p (j n c) -> p j n c", j=ST, n=DT)
    wq_r = wq.rearrange("p (n m) -> p n m", n=DT)
    wk_r = wk.rearrange("p (n m) -> p n m", n=DT)
    wv_r = wv.rearrange("p (n m) -> p n m", n=DT)
    wout_r = wout.rearrange("p (h e) -> p h e", h=HL)
    cos_r = cosf.rearrange("p (j c) -> p j c", j=ST)
    ssin_r = ssinf.rearrange("p (j c) -> p j c", j=ST)

    with tile.TileContext(nc) as tc:
        with (
            tc.tile_pool(name="const", bufs=1) as constp,
            tc.tile_pool(name="qkt", bufs=1) as qktp,
            tc.tile_pool(name="vpool", bufs=1) as vpool,
            tc.tile_pool(name="woutp", bufs=1) as woutp,
        ):
            # ---- constants ----
            ident = constp.tile([128, 128], f16, name="ident")
            make_identity(nc, ident)
            ones16 = constp.tile([128, 128], f16, name="ones16")
            nc.vector.memset(ones16, 1.0)
            # bias for rms' = sqrt(ssq/sqrt(C) + eps*sqrt(C)) = rms*C^0.25
            epsb = constp.tile([128, 1], f32, name="epsb")
            nc.vector.memset(epsb, EPS * SQRT_C)

            # qs/ks scale tiles: allocated here, DMA'd after the first
            # weight group so weight bytes hit the DMA pipe first
            qs_bc = constp.tile([128, C], f16, name="qs_bc")
            ks_bc = constp.tile([128, C], f16, name="ks_bc")
            qs_rot = constp.tile([128, C], f16, name="qs_rot")
            ks_rot = constp.tile([128, C], f16, name="ks_rot")

            qT = qktp.tile([128, HL, S], f16, name="qT")
            kT = qktp.tile([128, HL, S], f16, name="kT")
            v_sb = vpool.tile([128, ST, M], f16, name="v_sb")
            wout_sb = woutp.tile([128, HL, D], f16, name="wout_sb")

            # ---- phase 1: QKV projection + rmsnorm + rope + transpose ----
            with (
                tc.tile_pool(name="wqkv", bufs=1) as wqkvp,
                tc.tile_pool(name="rope", bufs=1) as ropep,
                tc.tile_pool(name="ph1", bufs=2) as ph1,
                tc.tile_pool(name="accps", bufs=6, space="PSUM") as accps,
                tc.tile_pool(name="tps", bufs=2, space="PSUM") as tps,
            ):
                wq_sb = wqkvp.tile([128, DT, M], f16, name="wq_sb")
                wk_sb = wqkvp.tile([128, DT, M], f16, name="wk_sb")
                wv_sb = wqkvp.tile([128, DT, M], f16, name="wv_sb")
                # DMA schedule: small first n-group so the first matmul can
                # start early; x tiles and rope tables interleaved; wout and
                # remaining x tiles stream during the j loop.
                WGRPS = [(0, 1), (1, 3), (3, 6), (6, 10), (10, 16)]
                for w_sb, w_r in ((wq_sb, wq_r), (wk_sb, wk_r), (wv_sb, wv_r)):
                    lo, hi = WGRPS[0]
                    nc.sync.dma_start(out=w_sb[:, lo:hi, :], in_=w_r[:, lo:hi, :])
                xjs = {}
                for j in (0, 1):
                    xj = ph1.tile(
                        [128, DT, 128], f16, name="xj", tag="xj", bufs=6
                    )
                    if j == 0:
                        # split so the first matmul gates on n=0..3 only
                        nc.sync.dma_start(
                            out=xj[:, 0:4, :], in_=xp_r[:, j, 0:4, :]
                        )
                        nc.sync.dma_start(
                            out=xj[:, 4:DT, :], in_=xp_r[:, j, 4:DT, :]
                        )
                    else:
                        nc.sync.dma_start(out=xj, in_=xp_r[:, j, :, :])
                    xjs[j] = xj
                for w_bc, w_dram in ((qs_bc, qs), (ks_bc, ks)):
                    src = bass.AP(
                        tensor=w_dram.ap().tensor, offset=0, ap=[[0, 128], [1, C]]
                    )
                    nc.sync.dma_start(out=w_bc, in_=src)
                for w_rot, w_bc in ((qs_rot, qs_bc), (ks_rot, ks_bc)):
                    nc.gpsimd.tensor_copy(
                        out=w_rot[:, 0 : C // 2], in_=w_bc[:, C // 2 : C]
                    )
                    nc.gpsimd.tensor_copy(
                        out=w_rot[:, C // 2 : C], in_=w_bc[:, 0 : C // 2]
                    )
                # PE warmup on resident constants while the weight DMAs
                # stream: ~5us of dummy matmuls gets HAM to 8/8 before the
                # first real matmul instead of paying the cold clock on it
                warm = accps.tile([128, 512], f32, name="warm", tag="acc")
                for _ in range(72):
                    nc.tensor.matmul(
                        warm[:, 0:128], lhsT=ident, rhs=ones16,
                        start=True, stop=True,
                    )
                # prime the ACT Sqrt table while ACT is idle so the first
                # rms sqrt doesn't eat a table load (Exp is primed at the
                # end of the j loop, just before phase 2 needs it)
                dummy = ph1.tile([1, 2], f16, name="dummy", bufs=1)
                nc.scalar.activation(out=dummy[:, 0:1], in_=epsb[0:1, :],
                                     func=Act.Sqrt)
                cos_t = ropep.tile([128, ST, C], f16, name="cos_t")
                ssin_t = ropep.tile([128, ST, C], f16, name="ssin_t")
                for gi, (lo, hi) in enumerate(WGRPS[1:]):
                    for w_sb, w_r in ((wq_sb, wq_r), (wk_sb, wk_r), (wv_sb, wv_r)):
                        nc.sync.dma_start(
                            out=w_sb[:, lo:hi, :], in_=w_r[:, lo:hi, :]
                        )
                    if gi == 0:
                        nc.sync.dma_start(out=cos_t, in_=cos_r)
                        nc.sync.dma_start(out=ssin_t, in_=ssin_r)

                pend_tr = []  # deferred transposes: (t1, dstT, j)

                def flush_transposes(upto_j=None):
                    while pend_tr and (
                        upto_j is None or pend_tr[0][2] <= upto_j
                    ):
                        t1, dstT, j = pend_tr.pop(0)
                        pt = tps.tile([128, M], f16, name="pt")
                        for h in range(HL):
                            nc.tensor.transpose(
                                pt[:, h * C : (h + 1) * C], t1[:, h, :], ident
                            )
                        nc.vector.tensor_copy(
                            out=dstT[:, :, j * 128 : (j + 1) * 128],
                            in_=pt.rearrange("p (a c) -> p a c", a=HL),
                        )

                for j in range(ST):
                    if j in xjs:
                        xj = xjs.pop(j)
                    else:
                        xj = ph1.tile(
                            [128, DT, 128], f16, name="xj", tag="xj", bufs=6
                        )
                        nc.sync.dma_start(out=xj, in_=xp_r[:, j, :, :])
                    if j == 3:
                        for h in range(HL):
                            nc.sync.dma_start(
                                out=wout_sb[:, h, :], in_=wout_r[:, h, :]
                            )
                    # rope tables for this j on GpSimd, ahead of the chain
                    ropes = {}
                    for key, tab, w_bc, w_rot in (
                        ("q", cos_t, qs_bc, qs_rot),
                        ("k", cos_t, ks_bc, ks_rot),
                    ):
                        cwF = ph1.tile([128, M], f16, name="cwF", bufs=4, tag="cwF")
                        nc.gpsimd.tensor_mul(
                            out=cwF.rearrange("p (a c) -> p a c", a=HL),
                            in0=_bcast_heads(cos_t[:, j, :], HL),
                            in1=_bcast_heads(w_bc, HL),
                        )
                        swF = ph1.tile([128, M], f16, name="swF", bufs=4, tag="swF")
                        nc.gpsimd.tensor_mul(
                            out=swF.rearrange("p (a c) -> p a c", a=HL),
                            in0=_bcast_heads(ssin_t[:, j, :], HL),
                            in1=_bcast_heads(w_rot, HL),
                        )
                        ropes[key] = (cwF, swF)
                    pq = accps.tile([128, M], f32, name="pq", tag="acc")
                    pk = accps.tile([128, M], f32, name="pk", tag="acc")
                    pv = accps.tile([128, M], f32, name="pv", tag="acc")
                    for n in range(DT):
                        if n == 12:
                            flush_transposes(upto_j=j - 2)
                        xsl = xj[:, n, :]
                        nc.tensor.matmul(
                            pq, lhsT=xsl, rhs=wq_sb[:, n, :],
                            start=(n == 0), stop=(n == DT - 1),
                        )
                        nc.tensor.matmul(
                            pk, lhsT=xsl, rhs=wk_sb[:, n, :],
                            start=(n == 0), stop=(n == DT - 1),
                        )
                        nc.tensor.matmul(
                            pv, lhsT=xsl, rhs=wv_sb[:, n, :],
                            start=(n == 0), stop=(n == DT - 1),
                        )
                    # v: plain copy to SBUF (cast fp16)
                    nc.scalar.copy(out=v_sb[:, j, :], in_=pv)
                    # q, k: rmsnorm scale + rope, all heads at once
                    for pacc, key, dstT in (
                        (pq, "q", qT),
                        (pk, "k", kT),
                    ):
                        cwF, swF = ropes[key]
                        xq = ph1.tile([128, M], f16, name="xq", bufs=4)
                        nc.scalar.copy(out=xq, in_=pacc)
                        # per-head sum of squares via ACT Square+accum_out
                        ssq4 = ph1.tile([128, HL], f32, name="ssq4", bufs=3)
                        sqd = ph1.tile([128, C], f16, name="sqd", bufs=2)
                        for h in range(HL):
                            nc.scalar.activation(
                                out=sqd, in_=xq[:, h * C : (h + 1) * C],
                                func=Act.Square,
                                accum_out=ssq4[:, h : h + 1],
                            )
                        rms4 = ph1.tile([128, HL], f32, name="rms4", bufs=3)
                        nc.scalar.activation(
                            out=rms4, in_=ssq4,
                            func=Act.Sqrt, scale=1.0 / SQRT_C, bias=epsb,
                        )
                        r4 = ph1.tile([128, HL], f16, name="r4", bufs=3)
                        with nc.allow_low_precision(
                            reason="1/rms fp16: 2e-4 rel on 2e-2 budget"
                        ):
                            nc.vector.reciprocal(out=r4, in_=rms4)
                        # per-head 1/(rms*C^0.25): one broadcast-AP multiply
                        xqs = ph1.tile([128, M], f16, name="xqs", bufs=3)
                        nc.vector.tensor_mul(
                            out=xqs.rearrange("p (a c) -> p a c", a=HL),
                            in0=xq.rearrange("p (a c) -> p a c", a=HL),
                            in1=_bcast_inner(r4, C),
                        )
                        # rotate_half
                        xqs3 = xqs.rearrange("p (a c) -> p a c", a=HL)
                        sh = ph1.tile([128, HL, C], f16, name="sh", bufs=3)
                        nc.vector.tensor_copy(
                            out=sh[:, :, 0 : C // 2], in_=xqs3[:, :, C // 2 : C]
                        )
                        nc.vector.tensor_copy(
                            out=sh[:, :, C // 2 : C], in_=xqs3[:, :, 0 : C // 2]
                        )
                        ta = ph1.tile([128, M], f16, name="ta", bufs=3)
                        shf = sh.rearrange("p a c -> p (a c)")
                        nc.vector.tensor_mul(out=ta, in0=xqs, in1=cwF)
                        nc.vector.tensor_mul(out=shf, in0=shf, in1=swF)
                        t1 = ph1.tile([128, HL, C], f16, name="t1", bufs=6)
                        nc.vector.tensor_add(
                            out=t1.rearrange("p a c -> p (a c)"), in0=ta, in1=shf
                        )
                        pend_tr.append((t1, dstT, j))
                    if j == ST - 1:
                        # preload the Exp/Ln table while the PE finishes the
                        # last block so phase 2's first exp starts clean
                        nc.scalar.activation(
                            out=dummy[:, 1:2], in_=epsb[0:1, :], func=Act.Exp
                        )
                flush_transposes()

            # ---- phase 2: attention + output projection ----
            with (
                tc.tile_pool(name="att", bufs=4) as attp,
                tc.tile_pool(name="esum", bufs=1) as esump,
                tc.tile_pool(name="outT", bufs=2) as outTp,
                tc.tile_pool(name="osb", bufs=1) as osbp,
                tc.tile_pool(name="scps", bufs=2, space="PSUM") as scps,
                tc.tile_pool(name="pod", bufs=3, space="PSUM") as podp,
                tc.tile_pool(name="prj", bufs=1, space="PSUM") as prjp,
            ):
                fillers = deque()   # pending proj-group emitters (PE filler)
                pending_fin = []    # heads awaiting denominator merge

                def emit_filler():
                    if fillers:
                        fillers.popleft()()

                def finish_head(fin):
                    """Merge the DVE+GpSimd partial e-sums into a
                    pre-broadcast PSUM denominator, then fast-reciprocal
                    and scale po into outT."""
                    po, esA, esB, outT_slice = fin
                    den = podp.tile([128, 512], f32, name="den", tag="den",
                                    bufs=1)
                    nc.tensor.matmul(
                        den, lhsT=ones16, rhs=esB, start=True, stop=False
                    )
                    nc.tensor.matmul(
                        den, lhsT=ones16, rhs=esA, start=False, stop=True
                    )
                    # 1/den on DVE: slow (~3.3us) but emitted in DVE's idle
                    # window (tp2, before the den adds start at tp3) and
                    # nothing downstream needs outT for >10us.  Keeping it
                    # off ACT matters: any non-exp ACT op delays exp(tp) and
                    # stalls the score-PSUM ring two t-pairs later.
                    rinv = attp.tile([128, 512], f16, name="rinv", bufs=2,
                                     tag="rinv")
                    with nc.allow_low_precision(
                        reason="1/denominator fp16: 5e-4 rel on 2e-2 budget"
                    ):
                        nc.vector.reciprocal(out=rinv, in_=den)
                    nc.vector.tensor_mul(out=outT_slice, in0=po, in1=rinv)

                def make_proj_tasks(outT_prev, nprev, drain=False):
                    for g in range(16):
                        jj, dc = g // 4, g % 4

                        def task(jj=jj, dc=dc, outT_prev=outT_prev,
                                 nprev=nprev, g=g, drain=drain):
                            # in the final drain the po/den ring is idle, so
                            # odd groups borrow it to pipeline 2-wide
                            if drain and g % 2 == 1:
                                psum_out = podp.tile(
                                    [128, 512], f32, name="psum_out",
                                    tag="po", bufs=2
                                )
                            else:
                                psum_out = prjp.tile(
                                    [128, 512], f32, name="psum_out", tag="pr"
                                )
                            for h in range(HL):
                                nc.tensor.matmul(
                                    psum_out,
                                    lhsT=outT_prev[
                                        :, h, jj * 128 : (jj + 1) * 128
                                    ],
                                    rhs=wout_sb[
                                        :, h, dc * 512 : (dc + 1) * 512
                                    ],
                                    start=(h == 0), stop=(h == HL - 1),
                                )
                            out_sb = osbp.tile(
                                [128, 512], f16, name="out_sb", bufs=6,
                                tag="osb"
                            )
                            # GpSimd cannot read PSUM; split casts ACT/DVE
                            # (2 each per head -- fits both engines' slack)
                            if g % 2 == 0:
                                nc.scalar.copy(out=out_sb, in_=psum_out)
                            else:
                                nc.vector.tensor_copy(out=out_sb, in_=psum_out)
                            srow = (nprev * 4 + jj) * 128
                            nc.sync.dma_start(
                                out=out[srow : srow + 128,
                                        dc * 512 : (dc + 1) * 512],
                                in_=out_sb,
                            )

                        fillers.append(task)

                def warm_pe(n, target=None):
                    """Dependency-free matmuls that bridge an unavoidable PE
                    wait: any idle gap drops the PE to the 1.2GHz p-state for
                    the next 3us, so a bridged wait is ~3x cheaper than a
                    gap.  Results go to a scratch psc slot or to `target`, a
                    psum region the next real matmul overwrites (start=True)."""
                    if target is None:
                        wrm = scps.tile([128, 1024], f32, name="wrm", tag="sc")
                        target = wrm
                    for _ in range(n):
                        nc.tensor.matmul(
                            target[:, 0:128], lhsT=ident, rhs=ones16,
                            start=True, stop=True,
                        )

                # bridge the qT/kT dependency wait at the phase boundary:
                # the first score matmul gates on the last j-block's rope
                # chain (~7us of ACT+DVE latency after the last QKV matmul)
                warm_pe(130)

                pend_pv = None  # (po, e2, t0, t1b) PV deferred one t-pair

                def emit_pv():
                    nonlocal pend_pv
                    if pend_pv is None:
                        return
                    po, e2, t0, t1b, h = pend_pv
                    pend_pv = None
                    nc.tensor.matmul(
                        po,
                        lhsT=v_sb[:, t0, h * C : (h + 1) * C],
                        rhs=e2[:, 0:512],
                        start=(t0 == 0), stop=False,
                    )
                    nc.tensor.matmul(
                        po,
                        lhsT=v_sb[:, t1b, h * C : (h + 1) * C],
                        rhs=e2[:, 512:1024],
                        start=False, stop=(t1b == ST - 1),
                    )

                for nchunk in range(NSC):
                    ssl = slice(nchunk * 512, (nchunk + 1) * 512)
                    outT_n = outTp.tile([128, HL, 512], f16, name="outT_n")
                    for h in range(HL):
                        po = podp.tile([128, 512], f32, name="po",
                                       tag="po", bufs=2)
                        esA = esump.tile([128, 512], f16, name="esA",
                                         bufs=2, tag="esA")
                        esB = esump.tile([128, 512], f16, name="esB",
                                         bufs=2, tag="esB")
                        for tp in range(ST // 2):
                            t0, t1b = 2 * tp, 2 * tp + 1
                            psc = scps.tile(
                                [128, 1024], f32, name="psc", tag="sc"
                            )
                            if nchunk == 0:
                                # no proj filler exists yet: pad the PE
                                # stream past exp's pace so the score ring
                                # never stalls (and the clock stays hot)
                                for half in (0, 1):
                                    nc.tensor.matmul(
                                        psc[:, half * 512 : half * 512 + 512],
                                        lhsT=ident, rhs=qT[:, h, ssl],
                                        start=True, stop=True,
                                    )
                            nc.tensor.matmul(
                                psc[:, 0:512],
                                lhsT=kT[:, h, t0 * 128 : (t0 + 1) * 128],
                                rhs=qT[:, h, ssl],
                                start=True, stop=True,
                            )
                            nc.tensor.matmul(
                                psc[:, 512:1024],
                                lhsT=kT[:, h, t1b * 128 : (t1b + 1) * 128],
                                rhs=qT[:, h, ssl],
                                start=True, stop=True,
                            )
                            # PV of the previous t-pair: exp already done,
                            # so the PE never waits on ACT here
                            emit_pv()
                            if tp == 2 and pending_fin:
                                finish_head(pending_fin.pop())
                            # filler slots: >=16 per chunk, none during h0:
                            # chunk n-1's h3 outT is merged at (h0, tp2) and
                            # its 3.3us DVE reciprocal only lands ~tp6, so
                            # an h0 filler would block the PE queue on its
                            # own input
                            if (tp == 7 and h == 0) or (
                                h > 0 and tp in (0, 1, 3, 5, 7)
                            ):
                                emit_filler()
                            e2 = attp.tile([128, 1024], f16, name="e2",
                                           bufs=4, tag="e2")
                            nc.scalar.activation(out=e2, in_=psc, func=Act.Exp)
                            pend_pv = (po, e2, t0, t1b, h)
                            # denominator partial sums off the PE:
                            # early t-pairs on GpSimd, late on DVE
                            if tp == 0:
                                nc.gpsimd.tensor_add(
                                    out=esB, in0=e2[:, 0:512],
                                    in1=e2[:, 512:1024],
                                )
                            elif tp in (1, 2):
                                nc.gpsimd.tensor_add(
                                    out=esB, in0=esB, in1=e2[:, 0:512]
                                )
                                nc.gpsimd.tensor_add(
                                    out=esB, in0=esB, in1=e2[:, 512:1024]
                                )
                            elif tp == 3:
                                nc.vector.tensor_add(
                                    out=esA, in0=e2[:, 0:512],
                                    in1=e2[:, 512:1024],
                                )
                            else:
                                nc.vector.tensor_add(
                                    out=esA, in0=esA, in1=e2[:, 0:512]
                                )
                                nc.vector.tensor_add(
                                    out=esA, in0=esA, in1=e2[:, 512:1024]
                                )
                        pending_fin.append((po, esA, esB, outT_n[:, h, :]))
                    make_proj_tasks(outT_n, nchunk, drain=(nchunk == NSC - 1))
                # drain: last PV, last head finish, last chunk's projection
                emit_pv()
                while pending_fin:
                    finish_head(pending_fin.pop())
                # the first drain group waits ~4us on h3's reciprocal chain
                warm_pe(60)
                while fillers:
                    emit_filler()
    if split_waits:
        _split_multi_waits(nc)
    return nc


_NC_CACHE = {}


def _get_nc():
    if "nc" not in _NC_CACHE:
        _NC_CACHE["nc"] = build_core_kernel()
    return _NC_CACHE["nc"]


def make_in_maps(x, rope_emb, Wq, Wk, Wv, Wout, q_scale, k_scale):
    freqs = rope_emb.reshape(S, C).astype(np.float64)
    cosf = np.cos(freqs)
    sf = np.sin(freqs)
    ssinf = np.concatenate([-sf[:, : C // 2], sf[:, C // 2 :]], axis=1)
    # [p, (j, c)] layout for the rope tables
    cos_p = np.ascontiguousarray(
        cosf.reshape(ST, 128, C).transpose(1, 0, 2).reshape(128, -1), np.float16
    )
    ssin_p = np.ascontiguousarray(
        ssinf.reshape(ST, 128, C).transpose(1, 0, 2).reshape(128, -1), np.float16
    )
    in_maps = []
    for c in range(NCORES):
        b, hg = c // 4, c % 4
        sl = slice(hg * M, (hg + 1) * M)
        x_b = np.asarray(x[:, b, :], np.float16)  # [S, D]
        # xp[p, j, n, sc] = x_b[j*128+sc, n*128+p]
        xp = np.ascontiguousarray(
            x_b.reshape(ST, 128, DT, 128).transpose(3, 0, 2, 1).reshape(128, -1)
        )
        def wlayout(Wl):  # Wl: [M, D] -> [p, (n, m)]
            return np.ascontiguousarray(
                Wl.T.reshape(DT, 128, M).transpose(1, 0, 2).reshape(128, -1),
                np.float16,
            )
        # wout[p, (h, e)] = Wout[:, sl][e, h*128+p]
        wout_l = np.ascontiguousarray(
            Wout[:, sl].T.reshape(HL, 128, D).transpose(1, 0, 2).reshape(128, -1),
            np.float16,
        )
        in_maps.append(
            {
                "xp": xp,
                "wq": wlayout(np.asarray(Wq[sl, :], np.float32)),
                "wk": wlayout(np.asarray(Wk[sl, :], np.float32)),
                "wv": wlayout(np.asarray(Wv[sl, :], np.float32)),
                "wout": wout_l,
                "cosf": cos_p,
                "ssinf": ssin_p,
                "qs": np.ascontiguousarray(q_scale, np.float16),
                "ks": np.ascontiguousarray(k_scale, np.float16),
            }
        )
    return in_maps


def kernel(x, rope_emb, Wq, Wk, Wv, Wout, q_scale, k_scale, **run_kwargs):
    in_maps = make_in_maps(
        np.asarray(x, np.float32),
        np.asarray(rope_emb, np.float32),
        np.asarray(Wq, np.float32),
        np.asarray(Wk, np.float32),
        np.asarray(Wv, np.float32),
        np.asarray(Wout, np.float32),
        np.asarray(q_scale, np.float32),
        np.asarray(k_scale, np.float32),
    )
    nc = _get_nc()
    res = run_bass_kernel_spmd(nc, in_maps, core_ids=list(range(NCORES)), **run_kwargs)
    out = np.zeros((S, B, D), dtype=np.float32)
    for c in range(NCORES):
        out[:, c // 4, :] += np.asarray(res.results[c]["out"], np.float32)
    if run_kwargs.get("trace"):
        kernel.last_result = res
    return out
